# revision 4
# baseline (speedup 1.0000x reference)
"""Two-layer GATv2 (heads=1) on 8 Trainium2 NeuronCores — v2.

Per core: nodes dst-sharded (6250/core, 50 blocks of 125). Edges grouped
128/group, batch-major: per batch of 5 blocks, [lo-src groups | hi-src
groups] (lo/hi split because dma_gather indices are int16).

Per group:
  - xl[src] via batched dma_gather (bf16, elem 256B)
  - v = onehot_ea^T @ xr_aug + I @ xlg accumulated in PSUM (bf16 matmuls,
    onehot+ea host-built in fp8, streamed from DRAM)
  - Prelu amortized over 4 (L1) / 8 (L2) groups per PSUM bank
  - logits via DVE STT with accum_out; exp once per batch
  - scatter via one-hot ups matmuls accumulating U|s per block in PSUM
Weights replicated; xl tables AllGathered (bf16) between phases.
"""
import sys

sys.path.insert(0, '/opt/trn_rl_repo')

import numpy as np
import ml_dtypes
from contextlib import ExitStack

import concourse.bass as bass
import concourse.tile as tile
import concourse.bacc as bacc
from concourse import mybir
from concourse.bass_utils import run_bass_kernel_spmd

F32 = mybir.dt.float32
BF16 = mybir.dt.bfloat16
FP8 = mybir.dt.float8e4
I16 = mybir.dt.int16
AF = mybir.ActivationFunctionType
ALU = mybir.AluOpType

NPBF16 = ml_dtypes.bfloat16
NPFP8 = ml_dtypes.float8_e4m3

NEG_SLOPE = 0.2

# Problem geometry (nn_Affinity_GAT_75557064671579)
N = 50000
E = 800000
F_IN = 128
H = 128
OUT = 64
NC = 8
NLOC = N // NC          # 6250
BLK = 125
NB = NLOC // BLK        # 50 blocks
BPB = 5                 # blocks per batch
NBAT = NB // BPB        # 10 batches
HALF = 32768            # int16 index split
PAD_DST = 999.0

LAST_EXEC_NS = [None]
LAST_RESULTS = [None]
LAST_RES = [None]


def prep_inputs(x, edge_index, edge_attr):
    """Host-side graph prep. Returns per-core input dicts + layout meta."""
    src = edge_index[0].astype(np.int64)
    dst = edge_index[1].astype(np.int64)
    ea = edge_attr[:, 0].astype(np.float32)

    # add_self_loops (fill_value='mean' over incoming edges)
    cnt = np.bincount(dst, minlength=N).astype(np.float32)
    asum = np.bincount(dst, weights=ea, minlength=N).astype(np.float32)
    loop_attr = asum / np.maximum(cnt, 1.0)
    src_all = np.concatenate([src, np.arange(N, dtype=np.int64)])
    dst_all = np.concatenate([dst, np.arange(N, dtype=np.int64)])
    ea_all = np.concatenate([ea, loop_attr])

    core_of = dst_all // NLOC
    per_core = []
    nlo = np.zeros((NC, NB), np.int64)
    nhi = np.zeros((NC, NB), np.int64)
    for c in range(NC):
        m = core_of == c
        s_c = src_all[m]
        d_c = dst_all[m] - c * NLOC
        a_c = ea_all[m]
        blk = d_c // BLK
        hi = (s_c >= HALF).astype(np.int64)
        key = blk * 2 + hi
        order = np.argsort(key, kind='stable')
        s_c, d_c, a_c, blk, hi, key = (s_c[order], d_c[order], a_c[order],
                                       blk[order], hi[order], key[order])
        # rank within each (blk, hi) bucket
        bc = np.bincount(key, minlength=NB * 2)
        starts = np.concatenate([[0], np.cumsum(bc)])[:-1]
        rank = np.arange(len(s_c)) - starts[key]
        per_core.append((s_c, d_c, a_c, blk, hi, rank))
        nlo[c] = bc[0::2]
        nhi[c] = bc[1::2]

    glo = np.maximum((nlo.max(axis=0) + 127) // 128, 0).astype(np.int64)
    ghi = np.maximum((nhi.max(axis=0) + 127) // 128, 0).astype(np.int64)

    # batch-major group numbering
    glo_bat = np.array([glo[k * BPB:(k + 1) * BPB].sum() for k in range(NBAT)])
    ghi_bat = np.array([ghi[k * BPB:(k + 1) * BPB].sum() for k in range(NBAT)])
    bg = glo_bat + ghi_bat
    bat_goff = np.concatenate([[0], np.cumsum(bg)])
    g_total = int(bat_goff[-1])
    bgmax = int(bg.max())

    # per-block rel group bases within batch
    lo_base = np.zeros(NB, np.int64)   # batch-relative
    hi_base = np.zeros(NB, np.int64)
    for k in range(NBAT):
        off = 0
        for b in range(k * BPB, (k + 1) * BPB):
            lo_base[b] = off
            off += glo[b]
        for b in range(k * BPB, (k + 1) * BPB):
            hi_base[b] = off
            off += ghi[b]

    # idx tensor column offsets (per batch: lo cols then hi cols; 16 idx/col)
    idx_off_lo = np.zeros(NBAT, np.int64)
    idx_off_hi = np.zeros(NBAT, np.int64)
    off = 0
    for k in range(NBAT):
        idx_off_lo[k] = off
        off += glo_bat[k] * 8
        idx_off_hi[k] = off
        off += ghi_bat[k] * 8
    idx_cols = int(off)

    meta = dict(glo=glo, ghi=ghi, glo_bat=glo_bat, ghi_bat=ghi_bat,
                bg=bg, bat_goff=bat_goff, g_total=g_total, bgmax=bgmax,
                lo_base=lo_base, hi_base=hi_base,
                idx_off_lo=idx_off_lo, idx_off_hi=idx_off_hi,
                idx_cols=idx_cols)

    core_inputs = []
    for c in range(NC):
        s_c, d_c, a_c, blk, hi, rank = per_core[c]
        k = blk // BPB
        grel = np.where(hi == 0, lo_base[blk], hi_base[blk]) + rank // 128
        gabs = bat_goff[k] + grel
        p = rank % 128

        onehot = np.zeros((128, g_total * 128), NPFP8)
        col = gabs * 128 + p
        onehot[(d_c - blk * BLK), col] = NPFP8(1.0)
        onehot[127, col] = a_c.astype(NPFP8)

        dstc = np.full((128, g_total), PAD_DST, np.float32)
        dstc[p, gabs] = (d_c - blk * BLK).astype(np.float32)

        idx_all = np.zeros((128, idx_cols), np.int16)
        # flat position within the batch's lo (or hi) idx list
        base_in_bat = np.where(hi == 0, lo_base[blk], hi_base[blk] - glo_bat[k])
        j = base_in_bat * 128 + rank
        colpos = np.where(hi == 0, idx_off_lo[k], idx_off_hi[k]) + j // 16
        val = np.where(hi == 0, s_c, s_c - HALF).astype(np.int16)
        idx_all[j % 16, colpos] = val
        # each of the 8 GPSIMD Q7 cores reads its own 16-partition stripe
        idx_all = np.tile(idx_all[0:16], (8, 1))

        xT = np.ascontiguousarray(
            x[c * NLOC:(c + 1) * NLOC].T).astype(NPBF16)
        core_inputs.append(dict(xT_bf=xT, onehot_d=onehot, dstc_d=dstc,
                                idx_d=idx_all))
    return core_inputs, meta


def build_program(meta):
    glo, ghi = meta['glo'], meta['ghi']
    glo_bat, ghi_bat = meta['glo_bat'], meta['ghi_bat']
    bg, bat_goff = meta['bg'], meta['bat_goff']
    g_total, bgmax = meta['g_total'], meta['bgmax']
    lo_base, hi_base = meta['lo_base'], meta['hi_base']
    idx_off_lo, idx_off_hi = meta['idx_off_lo'], meta['idx_off_hi']
    idx_cols = meta['idx_cols']

    nc = bacc.Bacc("TRN2", target_bir_lowering=False, debug=False,
                   num_devices=NC, num_swdge_queues=4)

    # --- DRAM I/O ---
    xT_bf = nc.dram_tensor("xT_bf", [F_IN, NLOC], BF16, kind="ExternalInput")
    onehot_d = nc.dram_tensor("onehot_d", [128, g_total * 128], FP8,
                              kind="ExternalInput")
    dstc_d = nc.dram_tensor("dstc_d", [128, g_total], F32, kind="ExternalInput")
    idx_d = nc.dram_tensor("idx_d", [128, idx_cols], I16, kind="ExternalInput")
    Wl1_d = nc.dram_tensor("Wl1b", [F_IN, H], BF16, kind="ExternalInput")
    Wr1_d = nc.dram_tensor("Wr1b", [F_IN, H], BF16, kind="ExternalInput")
    Wl2_d = nc.dram_tensor("Wl2b", [H, OUT], BF16, kind="ExternalInput")
    Wr2_d = nc.dram_tensor("Wr2b", [H, OUT], BF16, kind="ExternalInput")
    We1_d = nc.dram_tensor("We1t", [1, NB * H], BF16, kind="ExternalInput")
    We2_d = nc.dram_tensor("We2t", [1, NB * OUT], BF16, kind="ExternalInput")
    att1_d = nc.dram_tensor("att1b", [128, H], BF16, kind="ExternalInput")
    att2_d = nc.dram_tensor("att2b", [128, OUT], BF16, kind="ExternalInput")
    b1_d = nc.dram_tensor("b1b", [128, H], F32, kind="ExternalInput")
    b2_d = nc.dram_tensor("b2b", [128, OUT], F32, kind="ExternalInput")
    iota_d = nc.dram_tensor("iota_row", [128, 128], BF16, kind="ExternalInput")
    identb_d = nc.dram_tensor("identb", [128, 128], BF16, kind="ExternalInput")
    ident8_d = nc.dram_tensor("ident8", [128, 128], FP8, kind="ExternalInput")
    ones_d = nc.dram_tensor("ones_colb", [128, 1], BF16, kind="ExternalInput")
    out_slice = nc.dram_tensor("out_slice", [NLOC, OUT], F32,
                               kind="ExternalOutput")

    # internal DRAM
    ag1_in = nc.dram_tensor("ag1_in", [NLOC, H], BF16)
    xl1_full = nc.dram_tensor("xl1_full", [N, H], BF16, addr_space="Shared")
    ag2_in = nc.dram_tensor("ag2_in", [NLOC, 128], BF16)
    xl2_full = nc.dram_tensor("xl2_full", [N, 128], BF16, addr_space="Shared")

    with tile.TileContext(nc) as tc:
        with ExitStack() as ctx:
            cpool = ctx.enter_context(tc.tile_pool(name="const", bufs=1))
            mm_pool = ctx.enter_context(tc.tile_pool(name="mmx", bufs=3))
            mm_ps = ctx.enter_context(tc.tile_pool(name="mmps", bufs=2,
                                                   space="PSUM"))
            ps_t = ctx.enter_context(tc.tile_pool(name="pst", bufs=2,
                                                  space="PSUM"))
            ps_v = ctx.enter_context(tc.tile_pool(name="psv", bufs=2,
                                                  space="PSUM"))
            ps_u = ctx.enter_context(tc.tile_pool(name="psu", bufs=2,
                                                  space="PSUM"))
            z_pool = ctx.enter_context(tc.tile_pool(name="zp", bufs=2))
            scr_pool = ctx.enter_context(tc.tile_pool(name="scr", bufs=2))
            sw_pool = ctx.enter_context(tc.tile_pool(name="swp", bufs=3))
            np_pool = ctx.enter_context(tc.tile_pool(name="nodep", bufs=3))

            def cload(name, dram, shape, dt):
                t = cpool.tile(shape, dt, tag=name)
                nc.sync.dma_start(out=t[:], in_=dram[:])
                return t

            Wl1 = cload("Wl1", Wl1_d, [F_IN, H], BF16)
            Wr1 = cload("Wr1", Wr1_d, [F_IN, H], BF16)
            Wl2 = cload("Wl2", Wl2_d, [H, OUT], BF16)
            Wr2 = cload("Wr2", Wr2_d, [H, OUT], BF16)
            att1 = cload("att1", att1_d, [128, H], BF16)
            att2 = cload("att2", att2_d, [128, OUT], BF16)
            b1 = cload("b1", b1_d, [128, H], F32)
            b2 = cload("b2", b2_d, [128, OUT], F32)
            iota_row = cload("iota", iota_d, [128, 128], BF16)
            identb = cload("identb", identb_d, [128, 128], BF16)
            ident8 = cload("ident8", ident8_d, [128, 128], FP8)
            ones_colb = cload("onesc", ones_d, [128, 1], BF16)
            idx_sb = cpool.tile([128, idx_cols], I16, tag="idxsb")
            nc.sync.dma_start(out=idx_sb[:], in_=idx_d[:])

            # persistent xr tables (row 127 = We, rows 125:127 zero)
            xr1_sb = cpool.tile([128, NB * H], BF16, tag="xr1")
            nc.vector.memset(xr1_sb[:], 0.0)
            nc.sync.dma_start(out=xr1_sb[127:128, :], in_=We1_d[:])
            xr2_sb = cpool.tile([128, NB * OUT], BF16, tag="xr2")
            nc.vector.memset(xr2_sb[:], 0.0)
            nc.sync.dma_start(out=xr2_sb[127:128, :], in_=We2_d[:])

            # edge-phase streaming buffers
            xlg_bufs = [cpool.tile([128, bgmax, 128], BF16, tag=f"xlg{i}",
                                   name=f"xlg{i}") for i in range(3)]
            oh_bufs = [cpool.tile([128, bgmax * 128], FP8, tag=f"oh{i}",
                                  name=f"oh{i}") for i in range(2)]
            dst_bufs = [cpool.tile([128, bgmax], F32, tag=f"dstb{i}",
                                   name=f"dstb{i}") for i in range(3)]
            logit_bufs = [cpool.tile([128, bgmax], F32, tag=f"lg{i}",
                                     name=f"lg{i}") for i in range(2)]
            w_bufs = [cpool.tile([128, bgmax], F32, tag=f"wb{i}",
                                 name=f"wb{i}") for i in range(2)]

            qn = [0]

            # ============ layer 1 node matmul phase ============
            for j in range(NB):
                xT_t = mm_pool.tile([F_IN, BLK], BF16, tag="xTt")
                nc.sync.dma_start(out=xT_t[:],
                                  in_=xT_bf[:, j * BLK:(j + 1) * BLK])
                pa = mm_ps.tile([BLK, H], F32, tag="mm")
                nc.tensor.matmul(out=pa[:], lhsT=xT_t[:], rhs=Wl1[:],
                                 start=True, stop=True)
                sa = mm_pool.tile([BLK, H], BF16, tag="sa")
                nc.scalar.activation(sa[:], pa[:], AF.Copy)
                nc.sync.dma_start(out=ag1_in[j * BLK:(j + 1) * BLK, :],
                                  in_=sa[:])
                pb = mm_ps.tile([BLK, H], F32, tag="mm")
                nc.tensor.matmul(out=pb[:], lhsT=xT_t[:], rhs=Wr1[:],
                                 start=True, stop=True)
                nc.vector.tensor_copy(out=xr1_sb[0:BLK, j * H:(j + 1) * H],
                                      in_=pb[:])

            tc.strict_bb_all_engine_barrier()
            nc.gpsimd.collective_compute(
                "AllGather", ALU.bypass,
                replica_groups=[list(range(NC))],
                ins=[ag1_in[:]], outs=[xl1_full[:]])
            tc.strict_bb_all_engine_barrier()

            # ============ edge phase ============
            def edge_layer(F, table_d, xr_sb, att_t, b_t, sink, layer2):
                PR = 512 // F

                def emit_dmas(k):
                    g0 = int(bat_goff[k])
                    BG = int(bg[k])
                    glo_k = int(glo_bat[k])
                    ghi_k = int(ghi_bat[k])
                    oh = oh_bufs[k % 2]
                    nc.sync.dma_start(
                        out=oh[:, 0:BG * 128],
                        in_=onehot_d[:, g0 * 128:(g0 + BG) * 128])
                    db = dst_bufs[k % 3]
                    nc.sync.dma_start(out=db[:, 0:BG],
                                      in_=dstc_d[:, g0:g0 + BG])
                    xg = xlg_bufs[k % 3]
                    if glo_k > 0:
                        c0 = int(idx_off_lo[k])
                        nidx = glo_k * 128
                        nc.gpsimd.dma_gather(
                            xg[:, 0:glo_k, :], table_d[:],
                            idx_sb[:, c0:c0 + glo_k * 8],
                            nidx, nidx, 128,
                            queue_num=qn[0] % 4, single_packet=False)
                        qn[0] += 1
                    if ghi_k > 0:
                        c0 = int(idx_off_hi[k])
                        nidx = ghi_k * 128
                        nc.gpsimd.dma_gather(
                            xg[:, glo_k:glo_k + ghi_k, :],
                            table_d[HALF:N, :],
                            idx_sb[:, c0:c0 + ghi_k * 8],
                            nidx, nidx, 128,
                            queue_num=qn[0] % 4, single_packet=False)
                        qn[0] += 1

                def blk_of_group(k, grel):
                    """Map batch-relative group index -> block id."""
                    glo_k = int(glo_bat[k])
                    if grel < glo_k:
                        off = 0
                        for b in range(k * BPB, (k + 1) * BPB):
                            if grel < off + glo[b]:
                                return b
                            off += int(glo[b])
                    else:
                        off = glo_k
                        for b in range(k * BPB, (k + 1) * BPB):
                            if grel < off + ghi[b]:
                                return b
                            off += int(ghi[b])
                    raise AssertionError("bad group")

                def phase1_tiles(k):
                    """Generator: emits one PR-tile of phase1 per next()."""
                    BG = int(bg[k])
                    xg = xlg_bufs[k % 3]
                    oh = oh_bufs[k % 2]
                    lgb = logit_bufs[k % 2]
                    ntile = (BG + PR - 1) // PR
                    for t in range(ntile):
                        gA = t * PR
                        gZ = min(gA + PR, BG)
                        used = gZ - gA
                        # ONE accumulation group per PSUM bank-tile:
                        # start=True resets has_written BANK-WIDE, so only
                        # the first matmul starts; fresh regions write
                        # (hw=0), repeat writes accumulate (hw=1).
                        vt = ps_v.tile([128, 512], F32, tag="vt")
                        for i in range(used):
                            g = gA + i
                            b = blk_of_group(k, g)
                            nc.tensor.matmul(
                                out=vt[:, i * F:(i + 1) * F],
                                lhsT=oh[:, g * 128:(g + 1) * 128],
                                rhs=xr_sb[:, b * F:(b + 1) * F],
                                start=(i == 0), stop=False,
                                skip_group_check=True)
                            nc.tensor.matmul(
                                out=vt[:, i * F:(i + 1) * F],
                                lhsT=ident8[:],
                                rhs=xg[:, g, 0:F],
                                start=False, stop=(i == used - 1),
                                skip_group_check=True)
                        zt = z_pool.tile([128, 512], BF16, tag="zt")
                        nc.scalar.activation(zt[:, 0:used * F],
                                             vt[:, 0:used * F],
                                             AF.Prelu, alpha=NEG_SLOPE)
                        for i in range(used):
                            g = gA + i
                            scr = scr_pool.tile([128, F], BF16, tag="sc")
                            nc.vector.scalar_tensor_tensor(
                                out=scr[:], in0=zt[:, i * F:(i + 1) * F],
                                scalar=1.0, in1=att_t[:, 0:F],
                                op0=ALU.mult, op1=ALU.mult,
                                accum_out=lgb[:, g:g + 1])
                        yield

                def emit_exp(k):
                    BG = int(bg[k])
                    nc.scalar.activation(w_bufs[k % 2][:, 0:BG],
                                         logit_bufs[k % 2][:, 0:BG], AF.Exp)

                def phase2_block(k, b):
                    """ups accumulation + node phase + sink for block b."""
                    xg = xlg_bufs[k % 3]
                    db = dst_bufs[k % 3]
                    wb = w_bufs[k % 2]
                    glo_k = int(glo_bat[k])
                    groups = (list(range(int(lo_base[b]),
                                         int(lo_base[b] + glo[b]))) +
                              list(range(int(hi_base[b]),
                                         int(hi_base[b] + ghi[b]))))
                    # ONE accumulation group per bank: start only on the
                    # very first matmul; s column writes fresh (hw=0) then
                    # accumulates. L2 rhs 0:F+1 includes the table's ones
                    # col; L1 adds a separate N=1 s-matmul vs ones.
                    ups = ps_u.tile([128, F + 1], F32, tag="ups")
                    ng = len(groups)
                    for gi, g in enumerate(groups):
                        sw = sw_pool.tile([128, 128], BF16, tag="sw")
                        nc.vector.tensor_scalar(
                            out=sw[:], in0=iota_row[:],
                            scalar1=db[:, g:g + 1], scalar2=wb[:, g:g + 1],
                            op0=ALU.is_equal, op1=ALU.mult)
                        if layer2:
                            nc.tensor.matmul(out=ups[:, 0:F + 1], lhsT=sw[:],
                                             rhs=xg[:, g, 0:F + 1],
                                             start=(gi == 0),
                                             stop=(gi == ng - 1),
                                             skip_group_check=True)
                        else:
                            nc.tensor.matmul(out=ups[:, 0:F], lhsT=sw[:],
                                             rhs=xg[:, g, 0:F],
                                             start=(gi == 0), stop=False,
                                             skip_group_check=True)
                            nc.tensor.matmul(out=ups[:, F:F + 1], lhsT=sw[:],
                                             rhs=ones_colb[:],
                                             start=False,
                                             stop=(gi == ng - 1),
                                             skip_group_check=True)
                    # node phase: h = elu(U/s + b) = max(y, exp(min(y,0))-1)
                    rs = np_pool.tile([BLK, 1], F32, tag="rs")
                    nc.vector.reciprocal(rs[:], ups[0:BLK, F:F + 1])
                    y = np_pool.tile([BLK, F], BF16 if not layer2 else F32,
                                     tag="y")
                    nc.vector.scalar_tensor_tensor(
                        out=y[:], in0=ups[0:BLK, 0:F], scalar=rs[:],
                        in1=b_t[0:BLK, 0:F], op0=ALU.mult, op1=ALU.add)
                    mn = np_pool.tile([BLK, F], BF16 if not layer2 else F32,
                                      tag="mn")
                    nc.vector.tensor_scalar_min(mn[:], y[:], 0.0)
                    ex = np_pool.tile([BLK, F], BF16 if not layer2 else F32,
                                      tag="ex")
                    nc.scalar.activation(ex[:], mn[:], AF.Exp)
                    h2 = np_pool.tile([BLK, F], BF16 if not layer2 else F32,
                                      tag="h2")
                    nc.vector.scalar_tensor_tensor(
                        out=h2[:], in0=ex[:], scalar=-1.0, in1=y[:],
                        op0=ALU.add, op1=ALU.max)
                    sink(b, h2)

                # --- software pipeline over batches ---
                emit_dmas(0)
                prev_blocks = []
                for k in range(NBAT):
                    if k + 1 < NBAT:
                        emit_dmas(k + 1)
                    pending = list(prev_blocks)
                    tiles = phase1_tiles(k)
                    nt = (int(bg[k]) + PR - 1) // PR
                    ti = 0
                    for tile_i in tiles:
                        ti += 1
                        # interleave one phase2 block roughly every nt/BPB
                        if pending and ti % max(1, nt // BPB) == 0:
                            kb, bb = pending.pop(0)
                            phase2_block(kb, bb)
                    emit_exp(k)
                    for kb, bb in pending:
                        phase2_block(kb, bb)
                    prev_blocks = [(k, b)
                                   for b in range(k * BPB, (k + 1) * BPB)]
                for kb, bb in prev_blocks:
                    phase2_block(kb, bb)

            # ---- layer 1 ----
            def sink1(b, h2):
                hp = ps_t.tile([H, BLK], BF16, tag="tr")
                nc.tensor.matmul(out=hp[:], lhsT=h2[:, :],
                                 rhs=identb[0:BLK, 0:BLK], is_transpose=True)
                hs = np_pool.tile([H, BLK], BF16, tag="hs")
                nc.scalar.activation(hs[:], hp[:], AF.Copy)
                pa = mm_ps.tile([BLK, OUT], F32, tag="mm")
                nc.tensor.matmul(out=pa[:], lhsT=hs[:], rhs=Wl2[:],
                                 start=True, stop=True)
                sa = mm_pool.tile([BLK, OUT + 1], BF16, tag="sa2")
                nc.vector.memset(sa[:, OUT:OUT + 1], 1.0)
                nc.scalar.activation(sa[:, 0:OUT], pa[:], AF.Copy)
                nc.scalar.dma_start(
                    out=ag2_in[b * BLK:(b + 1) * BLK, 0:OUT + 1], in_=sa[:])
                pb = mm_ps.tile([BLK, OUT], F32, tag="mm")
                nc.tensor.matmul(out=pb[:], lhsT=hs[:], rhs=Wr2[:],
                                 start=True, stop=True)
                nc.vector.tensor_copy(
                    out=xr2_sb[0:BLK, b * OUT:(b + 1) * OUT], in_=pb[:])

            edge_layer(H, xl1_full, xr1_sb, att1, b1, sink1, layer2=False)

            tc.strict_bb_all_engine_barrier()
            nc.gpsimd.collective_compute(
                "AllGather", ALU.bypass,
                replica_groups=[list(range(NC))],
                ins=[ag2_in[:]], outs=[xl2_full[:]])
            tc.strict_bb_all_engine_barrier()

            # ---- layer 2 ----
            def sink2(b, h2):
                nc.scalar.dma_start(out=out_slice[b * BLK:(b + 1) * BLK, :],
                                    in_=h2[:])

            edge_layer(OUT, xl2_full, xr2_sb, att2, b2, sink2, layer2=True)

    nc.compile()
    return nc


_CACHE = {}


def kernel(x, edge_index, edge_attr, Wl1, Wr1, We1, att1, b1,
           Wl2, Wr2, We2, att2, b2, _trace=False):
    x = np.asarray(x, np.float32)
    edge_index = np.asarray(edge_index)
    edge_attr = np.asarray(edge_attr, np.float32)

    core_inputs, meta = prep_inputs(x, edge_index, edge_attr)

    key = (meta['g_total'], tuple(meta['glo'].tolist()),
           tuple(meta['ghi'].tolist()))
    if key not in _CACHE:
        _CACHE[key] = build_program(meta)
    nc = _CACHE[key]

    consts = dict(
        Wl1b=np.asarray(Wl1, np.float32).astype(NPBF16),
        Wr1b=np.asarray(Wr1, np.float32).astype(NPBF16),
        Wl2b=np.asarray(Wl2, np.float32).astype(NPBF16),
        Wr2b=np.asarray(Wr2, np.float32).astype(NPBF16),
        We1t=np.tile(np.asarray(We1, np.float32).reshape(1, H),
                     (1, NB)).astype(NPBF16),
        We2t=np.tile(np.asarray(We2, np.float32).reshape(1, OUT),
                     (1, NB)).astype(NPBF16),
        att1b=np.tile(np.asarray(att1, np.float32)[None, :],
                      (128, 1)).astype(NPBF16),
        att2b=np.tile(np.asarray(att2, np.float32)[None, :],
                      (128, 1)).astype(NPBF16),
        b1b=np.tile(np.asarray(b1, np.float32)[None, :], (128, 1)),
        b2b=np.tile(np.asarray(b2, np.float32)[None, :], (128, 1)),
        iota_row=np.tile(np.arange(128, dtype=np.float32)[None, :],
                         (128, 1)).astype(NPBF16),
        identb=np.eye(128, dtype=np.float32).astype(NPBF16),
        ident8=np.eye(128, dtype=np.float32).astype(NPFP8),
        ones_colb=np.ones((128, 1), NPBF16),
    )
    in_maps = []
    for c in range(NC):
        m = dict(consts)
        m.update(core_inputs[c])
        in_maps.append(m)

    res = run_bass_kernel_spmd(nc, in_maps, list(range(NC)), trace=_trace)
    LAST_EXEC_NS[0] = res.exec_time_ns
    LAST_RESULTS[0] = res.results
    LAST_RES[0] = res
    out = np.concatenate([res.results[c]["out_slice"] for c in range(NC)],
                         axis=0)
    return out


# revision 5
# speedup vs baseline: 1.1127x; 1.1127x over previous
"""Two-layer GATv2 (heads=1) on 8 Trainium2 NeuronCores — v2.

Per core: nodes dst-sharded (6250/core, 50 blocks of 125). Edges grouped
128/group, batch-major: per batch of 5 blocks, [lo-src groups | hi-src
groups] (lo/hi split because dma_gather indices are int16).

Per group:
  - xl[src] via batched dma_gather (bf16, elem 256B)
  - v = onehot_ea^T @ xr_aug + I @ xlg accumulated in PSUM (bf16 matmuls,
    onehot+ea host-built in fp8, streamed from DRAM)
  - Prelu amortized over 4 (L1) / 8 (L2) groups per PSUM bank
  - logits via DVE STT with accum_out; exp once per batch
  - scatter via one-hot ups matmuls accumulating U|s per block in PSUM
Weights replicated; xl tables AllGathered (bf16) between phases.
"""
import sys

sys.path.insert(0, '/opt/trn_rl_repo')

import numpy as np
import ml_dtypes
from contextlib import ExitStack

import concourse.bass as bass
import concourse.tile as tile
import concourse.bacc as bacc
from concourse import mybir
from concourse.bass_utils import run_bass_kernel_spmd

F32 = mybir.dt.float32
BF16 = mybir.dt.bfloat16
FP8 = mybir.dt.float8e4
I16 = mybir.dt.int16
AF = mybir.ActivationFunctionType
ALU = mybir.AluOpType

NPBF16 = ml_dtypes.bfloat16
NPFP8 = ml_dtypes.float8_e4m3

NEG_SLOPE = 0.2

# Problem geometry (nn_Affinity_GAT_75557064671579)
N = 50000
E = 800000
F_IN = 128
H = 128
OUT = 64
NC = 8
NLOC = N // NC          # 6250
BLK = 125
NB = NLOC // BLK        # 50 blocks
BPB = 5                 # blocks per batch
NBAT = NB // BPB        # 10 batches
HALF = 32768            # int16 index split
PAD_DST = 999.0

LAST_EXEC_NS = [None]
LAST_RESULTS = [None]
LAST_RES = [None]


def prep_inputs(x, edge_index, edge_attr):
    """Host-side graph prep. Returns per-core input dicts + layout meta."""
    src = edge_index[0].astype(np.int64)
    dst = edge_index[1].astype(np.int64)
    ea = edge_attr[:, 0].astype(np.float32)

    # add_self_loops (fill_value='mean' over incoming edges)
    cnt = np.bincount(dst, minlength=N).astype(np.float32)
    asum = np.bincount(dst, weights=ea, minlength=N).astype(np.float32)
    loop_attr = asum / np.maximum(cnt, 1.0)
    src_all = np.concatenate([src, np.arange(N, dtype=np.int64)])
    dst_all = np.concatenate([dst, np.arange(N, dtype=np.int64)])
    ea_all = np.concatenate([ea, loop_attr])

    core_of = dst_all // NLOC
    per_core = []
    nlo = np.zeros((NC, NB), np.int64)
    nhi = np.zeros((NC, NB), np.int64)
    for c in range(NC):
        m = core_of == c
        s_c = src_all[m]
        d_c = dst_all[m] - c * NLOC
        a_c = ea_all[m]
        blk = d_c // BLK
        hi = (s_c >= HALF).astype(np.int64)
        key = blk * 2 + hi
        order = np.argsort(key, kind='stable')
        s_c, d_c, a_c, blk, hi, key = (s_c[order], d_c[order], a_c[order],
                                       blk[order], hi[order], key[order])
        # rank within each (blk, hi) bucket
        bc = np.bincount(key, minlength=NB * 2)
        starts = np.concatenate([[0], np.cumsum(bc)])[:-1]
        rank = np.arange(len(s_c)) - starts[key]
        per_core.append((s_c, d_c, a_c, blk, hi, rank))
        nlo[c] = bc[0::2]
        nhi[c] = bc[1::2]

    glo = np.maximum((nlo.max(axis=0) + 127) // 128, 0).astype(np.int64)
    ghi = np.maximum((nhi.max(axis=0) + 127) // 128, 0).astype(np.int64)

    # batch-major group numbering
    glo_bat = np.array([glo[k * BPB:(k + 1) * BPB].sum() for k in range(NBAT)])
    ghi_bat = np.array([ghi[k * BPB:(k + 1) * BPB].sum() for k in range(NBAT)])
    bg = glo_bat + ghi_bat
    bat_goff = np.concatenate([[0], np.cumsum(bg)])
    g_total = int(bat_goff[-1])
    bgmax = int(bg.max())

    # per-block rel group bases within batch
    lo_base = np.zeros(NB, np.int64)   # batch-relative
    hi_base = np.zeros(NB, np.int64)
    for k in range(NBAT):
        off = 0
        for b in range(k * BPB, (k + 1) * BPB):
            lo_base[b] = off
            off += glo[b]
        for b in range(k * BPB, (k + 1) * BPB):
            hi_base[b] = off
            off += ghi[b]

    # idx tensor column offsets (per batch: lo cols then hi cols; 16 idx/col)
    idx_off_lo = np.zeros(NBAT, np.int64)
    idx_off_hi = np.zeros(NBAT, np.int64)
    off = 0
    for k in range(NBAT):
        idx_off_lo[k] = off
        off += glo_bat[k] * 8
        idx_off_hi[k] = off
        off += ghi_bat[k] * 8
    idx_cols = int(off)

    meta = dict(glo=glo, ghi=ghi, glo_bat=glo_bat, ghi_bat=ghi_bat,
                bg=bg, bat_goff=bat_goff, g_total=g_total, bgmax=bgmax,
                lo_base=lo_base, hi_base=hi_base,
                idx_off_lo=idx_off_lo, idx_off_hi=idx_off_hi,
                idx_cols=idx_cols)

    core_inputs = []
    for c in range(NC):
        s_c, d_c, a_c, blk, hi, rank = per_core[c]
        k = blk // BPB
        grel = np.where(hi == 0, lo_base[blk], hi_base[blk]) + rank // 128
        gabs = bat_goff[k] + grel
        p = rank % 128

        onehot = np.zeros((128, g_total * 128), NPFP8)
        col = gabs * 128 + p
        onehot[(d_c - blk * BLK), col] = NPFP8(1.0)
        onehot[127, col] = a_c.astype(NPFP8)

        onehotT = np.zeros((128, g_total * 128), NPFP8)
        onehotT[p, gabs * 128 + (d_c - blk * BLK)] = NPFP8(1.0)

        idx_all = np.zeros((128, idx_cols), np.int16)
        # flat position within the batch's lo (or hi) idx list
        base_in_bat = np.where(hi == 0, lo_base[blk], hi_base[blk] - glo_bat[k])
        j = base_in_bat * 128 + rank
        colpos = np.where(hi == 0, idx_off_lo[k], idx_off_hi[k]) + j // 16
        val = np.where(hi == 0, s_c, s_c - HALF).astype(np.int16)
        idx_all[j % 16, colpos] = val
        # each of the 8 GPSIMD Q7 cores reads its own 16-partition stripe
        idx_all = np.tile(idx_all[0:16], (8, 1))

        xT = np.ascontiguousarray(
            x[c * NLOC:(c + 1) * NLOC].T).astype(NPBF16)
        core_inputs.append(dict(xT_bf=xT, onehot_d=onehot, onehotT_d=onehotT,
                                idx_d=idx_all))
    return core_inputs, meta


def build_program(meta):
    glo, ghi = meta['glo'], meta['ghi']
    glo_bat, ghi_bat = meta['glo_bat'], meta['ghi_bat']
    bg, bat_goff = meta['bg'], meta['bat_goff']
    g_total, bgmax = meta['g_total'], meta['bgmax']
    lo_base, hi_base = meta['lo_base'], meta['hi_base']
    idx_off_lo, idx_off_hi = meta['idx_off_lo'], meta['idx_off_hi']
    idx_cols = meta['idx_cols']

    nc = bacc.Bacc("TRN2", target_bir_lowering=False, debug=False,
                   num_devices=NC, num_swdge_queues=4)

    # --- DRAM I/O ---
    xT_bf = nc.dram_tensor("xT_bf", [F_IN, NLOC], BF16, kind="ExternalInput")
    onehot_d = nc.dram_tensor("onehot_d", [128, g_total * 128], FP8,
                              kind="ExternalInput")
    onehotT_d = nc.dram_tensor("onehotT_d", [128, g_total * 128], FP8,
                               kind="ExternalInput")
    idx_d = nc.dram_tensor("idx_d", [128, idx_cols], I16, kind="ExternalInput")
    Wl1_d = nc.dram_tensor("Wl1b", [F_IN, H], BF16, kind="ExternalInput")
    Wr1_d = nc.dram_tensor("Wr1b", [F_IN, H], BF16, kind="ExternalInput")
    Wl2_d = nc.dram_tensor("Wl2b", [H, OUT], BF16, kind="ExternalInput")
    Wr2_d = nc.dram_tensor("Wr2b", [H, OUT], BF16, kind="ExternalInput")
    We1_d = nc.dram_tensor("We1t", [1, NB * H], BF16, kind="ExternalInput")
    We2_d = nc.dram_tensor("We2t", [1, NB * OUT], BF16, kind="ExternalInput")
    att1_d = nc.dram_tensor("att1b", [128, H], BF16, kind="ExternalInput")
    att2_d = nc.dram_tensor("att2b", [128, OUT], BF16, kind="ExternalInput")
    b1_d = nc.dram_tensor("b1b", [128, H], F32, kind="ExternalInput")
    b2_d = nc.dram_tensor("b2b", [128, OUT], F32, kind="ExternalInput")
    iota_d = nc.dram_tensor("iota_row", [128, 128], BF16, kind="ExternalInput")
    identb_d = nc.dram_tensor("identb", [128, 128], BF16, kind="ExternalInput")
    ident8_d = nc.dram_tensor("ident8", [128, 128], FP8, kind="ExternalInput")
    ones_d = nc.dram_tensor("ones_colb", [128, 1], BF16, kind="ExternalInput")
    out_slice = nc.dram_tensor("out_slice", [NLOC, OUT], F32,
                               kind="ExternalOutput")

    # internal DRAM
    ag1_in = nc.dram_tensor("ag1_in", [NLOC, H], BF16)
    xl1_full = nc.dram_tensor("xl1_full", [N, H], BF16, addr_space="Shared")
    ag2_in = nc.dram_tensor("ag2_in", [NLOC, 128], BF16)
    xl2_full = nc.dram_tensor("xl2_full", [N, 128], BF16, addr_space="Shared")

    with tile.TileContext(nc) as tc:
        with ExitStack() as ctx:
            cpool = ctx.enter_context(tc.tile_pool(name="const", bufs=1))
            mm_pool = ctx.enter_context(tc.tile_pool(name="mmx", bufs=3))
            mm_ps = ctx.enter_context(tc.tile_pool(name="mmps", bufs=2,
                                                   space="PSUM"))
            ps_t = ctx.enter_context(tc.tile_pool(name="pst", bufs=2,
                                                  space="PSUM"))
            ps_v = ctx.enter_context(tc.tile_pool(name="psv", bufs=2,
                                                  space="PSUM"))
            ps_u = ctx.enter_context(tc.tile_pool(name="psu", bufs=2,
                                                  space="PSUM"))
            z_pool = ctx.enter_context(tc.tile_pool(name="zp", bufs=2))
            scr_pool = ctx.enter_context(tc.tile_pool(name="scr", bufs=2))
            sw_pool = ctx.enter_context(tc.tile_pool(name="swp", bufs=3))
            np_pool = ctx.enter_context(tc.tile_pool(name="nodep", bufs=3))

            def cload(name, dram, shape, dt):
                t = cpool.tile(shape, dt, tag=name)
                nc.sync.dma_start(out=t[:], in_=dram[:])
                return t

            Wl1 = cload("Wl1", Wl1_d, [F_IN, H], BF16)
            Wr1 = cload("Wr1", Wr1_d, [F_IN, H], BF16)
            Wl2 = cload("Wl2", Wl2_d, [H, OUT], BF16)
            Wr2 = cload("Wr2", Wr2_d, [H, OUT], BF16)
            att1 = cload("att1", att1_d, [128, H], BF16)
            att2 = cload("att2", att2_d, [128, OUT], BF16)
            b1 = cload("b1", b1_d, [128, H], F32)
            b2 = cload("b2", b2_d, [128, OUT], F32)
            iota_row = cload("iota", iota_d, [128, 128], BF16)
            identb = cload("identb", identb_d, [128, 128], BF16)
            ident8 = cload("ident8", ident8_d, [128, 128], FP8)
            ones_colb = cload("onesc", ones_d, [128, 1], BF16)
            idx_sb = cpool.tile([128, idx_cols], I16, tag="idxsb")
            nc.sync.dma_start(out=idx_sb[:], in_=idx_d[:])

            # persistent xr tables (row 127 = We, rows 125:127 zero)
            xr1_sb = cpool.tile([128, NB * H], BF16, tag="xr1")
            nc.vector.memset(xr1_sb[:], 0.0)
            nc.sync.dma_start(out=xr1_sb[127:128, :], in_=We1_d[:])
            xr2_sb = cpool.tile([128, NB * OUT], BF16, tag="xr2")
            nc.vector.memset(xr2_sb[:], 0.0)
            nc.sync.dma_start(out=xr2_sb[127:128, :], in_=We2_d[:])

            # edge-phase streaming buffers
            xlg_bufs = [cpool.tile([128, bgmax, 128], BF16, tag=f"xlg{i}",
                                   name=f"xlg{i}") for i in range(3)]
            oh_bufs = [cpool.tile([128, bgmax * 128], FP8, tag=f"oh{i}",
                                  name=f"oh{i}") for i in range(2)]
            ohT_bufs = [cpool.tile([128, bgmax * 128], FP8, tag=f"ohT{i}",
                                   name=f"ohT{i}") for i in range(3)]
            logit_bufs = [cpool.tile([128, bgmax], F32, tag=f"lg{i}",
                                     name=f"lg{i}") for i in range(2)]
            w_bufs = [cpool.tile([128, bgmax], F32, tag=f"wb{i}",
                                 name=f"wb{i}") for i in range(2)]
            wbf_bufs = [cpool.tile([128, bgmax], BF16, tag=f"wf{i}",
                                   name=f"wf{i}") for i in range(2)]

            qn = [0]

            # ============ layer 1 node matmul phase ============
            for j in range(NB):
                xT_t = mm_pool.tile([F_IN, BLK], BF16, tag="xTt")
                nc.sync.dma_start(out=xT_t[:],
                                  in_=xT_bf[:, j * BLK:(j + 1) * BLK])
                pa = mm_ps.tile([BLK, H], F32, tag="mm")
                nc.tensor.matmul(out=pa[:], lhsT=xT_t[:], rhs=Wl1[:],
                                 start=True, stop=True)
                sa = mm_pool.tile([BLK, H], BF16, tag="sa")
                nc.scalar.activation(sa[:], pa[:], AF.Copy)
                nc.sync.dma_start(out=ag1_in[j * BLK:(j + 1) * BLK, :],
                                  in_=sa[:])
                pb = mm_ps.tile([BLK, H], F32, tag="mm")
                nc.tensor.matmul(out=pb[:], lhsT=xT_t[:], rhs=Wr1[:],
                                 start=True, stop=True)
                nc.vector.tensor_copy(out=xr1_sb[0:BLK, j * H:(j + 1) * H],
                                      in_=pb[:])

            tc.strict_bb_all_engine_barrier()
            nc.gpsimd.collective_compute(
                "AllGather", ALU.bypass,
                replica_groups=[list(range(NC))],
                ins=[ag1_in[:]], outs=[xl1_full[:]])
            tc.strict_bb_all_engine_barrier()

            # ============ edge phase ============
            def edge_layer(F, table_d, xr_sb, att_t, b_t, sink, layer2):
                PR = 512 // F

                def emit_dmas(k):
                    g0 = int(bat_goff[k])
                    BG = int(bg[k])
                    glo_k = int(glo_bat[k])
                    ghi_k = int(ghi_bat[k])
                    oh = oh_bufs[k % 2]
                    nc.sync.dma_start(
                        out=oh[:, 0:BG * 128],
                        in_=onehot_d[:, g0 * 128:(g0 + BG) * 128])
                    ohT = ohT_bufs[k % 3]
                    nc.scalar.dma_start(
                        out=ohT[:, 0:BG * 128],
                        in_=onehotT_d[:, g0 * 128:(g0 + BG) * 128])
                    xg = xlg_bufs[k % 3]
                    # each gather split in two halves on separate SWDGE
                    # queues: Q7 emission parallelizes across queues
                    q = 0
                    for base, n_g, coff, tab in (
                            (0, glo_k, int(idx_off_lo[k]), table_d[:]),
                            (glo_k, ghi_k, int(idx_off_hi[k]),
                             table_d[HALF:N, :])):
                        if n_g == 0:
                            continue
                        h = (n_g + 1) // 2
                        for a, z in ((0, h), (h, n_g)):
                            if z <= a:
                                continue
                            nidx = (z - a) * 128
                            nc.gpsimd.dma_gather(
                                xg[:, base + a:base + z, :], tab,
                                idx_sb[:, coff + a * 8:coff + z * 8],
                                nidx, nidx, 128,
                                queue_num=q % 4, single_packet=False)
                            q += 1

                def blk_of_group(k, grel):
                    """Map batch-relative group index -> block id."""
                    glo_k = int(glo_bat[k])
                    if grel < glo_k:
                        off = 0
                        for b in range(k * BPB, (k + 1) * BPB):
                            if grel < off + glo[b]:
                                return b
                            off += int(glo[b])
                    else:
                        off = glo_k
                        for b in range(k * BPB, (k + 1) * BPB):
                            if grel < off + ghi[b]:
                                return b
                            off += int(ghi[b])
                    raise AssertionError("bad group")

                def phase1_tiles(k):
                    """Generator: emits one PR-tile of phase1 per next()."""
                    BG = int(bg[k])
                    xg = xlg_bufs[k % 3]
                    oh = oh_bufs[k % 2]
                    lgb = logit_bufs[k % 2]
                    ntile = (BG + PR - 1) // PR
                    for t in range(ntile):
                        gA = t * PR
                        gZ = min(gA + PR, BG)
                        used = gZ - gA
                        # ONE accumulation group per PSUM bank-tile:
                        # start=True resets has_written BANK-WIDE, so only
                        # the first matmul starts; fresh regions write
                        # (hw=0), repeat writes accumulate (hw=1).
                        vt = ps_v.tile([128, 512], F32, tag="vt")
                        for i in range(used):
                            g = gA + i
                            b = blk_of_group(k, g)
                            nc.tensor.matmul(
                                out=vt[:, i * F:(i + 1) * F],
                                lhsT=oh[:, g * 128:(g + 1) * 128],
                                rhs=xr_sb[:, b * F:(b + 1) * F],
                                start=(i == 0), stop=False,
                                skip_group_check=True)
                            nc.tensor.matmul(
                                out=vt[:, i * F:(i + 1) * F],
                                lhsT=ident8[:],
                                rhs=xg[:, g, 0:F],
                                start=False, stop=(i == used - 1),
                                skip_group_check=True)
                        zt = z_pool.tile([128, 512], BF16, tag="zt")
                        nc.scalar.activation(zt[:, 0:used * F],
                                             vt[:, 0:used * F],
                                             AF.Prelu, alpha=NEG_SLOPE)
                        for i in range(used):
                            g = gA + i
                            scr = scr_pool.tile([128, F], BF16, tag="sc")
                            nc.vector.scalar_tensor_tensor(
                                out=scr[:], in0=zt[:, i * F:(i + 1) * F],
                                scalar=1.0, in1=att_t[:, 0:F],
                                op0=ALU.mult, op1=ALU.mult,
                                accum_out=lgb[:, g:g + 1])
                        yield

                def emit_exp(k):
                    BG = int(bg[k])
                    nc.scalar.activation(w_bufs[k % 2][:, 0:BG],
                                         logit_bufs[k % 2][:, 0:BG], AF.Exp)
                    nc.scalar.activation(wbf_bufs[k % 2][:, 0:BG],
                                         w_bufs[k % 2][:, 0:BG], AF.Copy)

                def phase2_block(k, b):
                    """ups accumulation + node phase + sink for block b."""
                    xg = xlg_bufs[k % 3]
                    ohT = ohT_bufs[k % 3]
                    wb = w_bufs[k % 2]
                    wf = wbf_bufs[k % 2]
                    glo_k = int(glo_bat[k])
                    groups = (list(range(int(lo_base[b]),
                                         int(lo_base[b] + glo[b]))) +
                              list(range(int(hi_base[b]),
                                         int(hi_base[b] + ghi[b]))))
                    # ONE accumulation group per bank: start only on the
                    # very first matmul; s column writes fresh (hw=0) then
                    # accumulates. L2 rhs 0:F+1 includes the table's ones
                    # col; L1 adds a separate N=1 s-matmul vs ones.
                    ups = ps_u.tile([128, F + 1], F32, tag="ups")
                    ng = len(groups)
                    for gi, g in enumerate(groups):
                        xlgw = sw_pool.tile([128, F], BF16, tag="xw")
                        nc.scalar.activation(xlgw[:], xg[:, g, 0:F], AF.Copy,
                                             scale=wb[:, g:g + 1])
                        nc.tensor.matmul(out=ups[:, 0:F],
                                         lhsT=ohT[:, g * 128:(g + 1) * 128],
                                         rhs=xlgw[:],
                                         start=(gi == 0), stop=False,
                                         skip_group_check=True)
                        nc.tensor.matmul(out=ups[:, F:F + 1],
                                         lhsT=ohT[:, g * 128:(g + 1) * 128],
                                         rhs=wf[:, g:g + 1],
                                         start=False, stop=(gi == ng - 1),
                                         skip_group_check=True)
                    # node phase: h = elu(U/s + b) = max(y, exp(min(y,0))-1)
                    rs = np_pool.tile([BLK, 1], F32, tag="rs")
                    nc.vector.reciprocal(rs[:], ups[0:BLK, F:F + 1])
                    y = np_pool.tile([BLK, F], BF16 if not layer2 else F32,
                                     tag="y")
                    nc.vector.scalar_tensor_tensor(
                        out=y[:], in0=ups[0:BLK, 0:F], scalar=rs[:],
                        in1=b_t[0:BLK, 0:F], op0=ALU.mult, op1=ALU.add)
                    mn = np_pool.tile([BLK, F], BF16 if not layer2 else F32,
                                      tag="mn")
                    nc.vector.tensor_scalar_min(mn[:], y[:], 0.0)
                    ex = np_pool.tile([BLK, F], BF16 if not layer2 else F32,
                                      tag="ex")
                    nc.scalar.activation(ex[:], mn[:], AF.Exp)
                    h2 = np_pool.tile([BLK, F], BF16 if not layer2 else F32,
                                      tag="h2")
                    nc.vector.scalar_tensor_tensor(
                        out=h2[:], in0=ex[:], scalar=-1.0, in1=y[:],
                        op0=ALU.add, op1=ALU.max)
                    sink(b, h2)

                # --- software pipeline over batches ---
                emit_dmas(0)
                prev_blocks = []
                for k in range(NBAT):
                    if k + 1 < NBAT:
                        emit_dmas(k + 1)
                    pending = list(prev_blocks)
                    tiles = phase1_tiles(k)
                    nt = (int(bg[k]) + PR - 1) // PR
                    ti = 0
                    for tile_i in tiles:
                        ti += 1
                        # interleave one phase2 block roughly every nt/BPB
                        if pending and ti % max(1, nt // BPB) == 0:
                            kb, bb = pending.pop(0)
                            phase2_block(kb, bb)
                    emit_exp(k)
                    for kb, bb in pending:
                        phase2_block(kb, bb)
                    prev_blocks = [(k, b)
                                   for b in range(k * BPB, (k + 1) * BPB)]
                for kb, bb in prev_blocks:
                    phase2_block(kb, bb)

            # ---- layer 1 ----
            def sink1(b, h2):
                hp = ps_t.tile([H, BLK], BF16, tag="tr")
                nc.tensor.matmul(out=hp[:], lhsT=h2[:, :],
                                 rhs=identb[0:BLK, 0:BLK], is_transpose=True)
                hs = np_pool.tile([H, BLK], BF16, tag="hs")
                nc.scalar.activation(hs[:], hp[:], AF.Copy)
                pa = mm_ps.tile([BLK, OUT], F32, tag="mm")
                nc.tensor.matmul(out=pa[:], lhsT=hs[:], rhs=Wl2[:],
                                 start=True, stop=True)
                sa = mm_pool.tile([BLK, OUT + 1], BF16, tag="sa2")
                nc.vector.memset(sa[:, OUT:OUT + 1], 1.0)
                nc.scalar.activation(sa[:, 0:OUT], pa[:], AF.Copy)
                nc.scalar.dma_start(
                    out=ag2_in[b * BLK:(b + 1) * BLK, 0:OUT + 1], in_=sa[:])
                pb = mm_ps.tile([BLK, OUT], F32, tag="mm")
                nc.tensor.matmul(out=pb[:], lhsT=hs[:], rhs=Wr2[:],
                                 start=True, stop=True)
                nc.vector.tensor_copy(
                    out=xr2_sb[0:BLK, b * OUT:(b + 1) * OUT], in_=pb[:])

            edge_layer(H, xl1_full, xr1_sb, att1, b1, sink1, layer2=False)

            tc.strict_bb_all_engine_barrier()
            nc.gpsimd.collective_compute(
                "AllGather", ALU.bypass,
                replica_groups=[list(range(NC))],
                ins=[ag2_in[:]], outs=[xl2_full[:]])
            tc.strict_bb_all_engine_barrier()

            # ---- layer 2 ----
            def sink2(b, h2):
                nc.scalar.dma_start(out=out_slice[b * BLK:(b + 1) * BLK, :],
                                    in_=h2[:])

            edge_layer(OUT, xl2_full, xr2_sb, att2, b2, sink2, layer2=True)

    nc.compile()
    return nc


_CACHE = {}


def kernel(x, edge_index, edge_attr, Wl1, Wr1, We1, att1, b1,
           Wl2, Wr2, We2, att2, b2, _trace=False):
    x = np.asarray(x, np.float32)
    edge_index = np.asarray(edge_index)
    edge_attr = np.asarray(edge_attr, np.float32)

    core_inputs, meta = prep_inputs(x, edge_index, edge_attr)

    key = (meta['g_total'], tuple(meta['glo'].tolist()),
           tuple(meta['ghi'].tolist()))
    if key not in _CACHE:
        _CACHE[key] = build_program(meta)
    nc = _CACHE[key]

    consts = dict(
        Wl1b=np.asarray(Wl1, np.float32).astype(NPBF16),
        Wr1b=np.asarray(Wr1, np.float32).astype(NPBF16),
        Wl2b=np.asarray(Wl2, np.float32).astype(NPBF16),
        Wr2b=np.asarray(Wr2, np.float32).astype(NPBF16),
        We1t=np.tile(np.asarray(We1, np.float32).reshape(1, H),
                     (1, NB)).astype(NPBF16),
        We2t=np.tile(np.asarray(We2, np.float32).reshape(1, OUT),
                     (1, NB)).astype(NPBF16),
        att1b=np.tile(np.asarray(att1, np.float32)[None, :],
                      (128, 1)).astype(NPBF16),
        att2b=np.tile(np.asarray(att2, np.float32)[None, :],
                      (128, 1)).astype(NPBF16),
        b1b=np.tile(np.asarray(b1, np.float32)[None, :], (128, 1)),
        b2b=np.tile(np.asarray(b2, np.float32)[None, :], (128, 1)),
        iota_row=np.tile(np.arange(128, dtype=np.float32)[None, :],
                         (128, 1)).astype(NPBF16),
        identb=np.eye(128, dtype=np.float32).astype(NPBF16),
        ident8=np.eye(128, dtype=np.float32).astype(NPFP8),
        ones_colb=np.ones((128, 1), NPBF16),
    )
    in_maps = []
    for c in range(NC):
        m = dict(consts)
        m.update(core_inputs[c])
        in_maps.append(m)

    res = run_bass_kernel_spmd(nc, in_maps, list(range(NC)), trace=_trace)
    LAST_EXEC_NS[0] = res.exec_time_ns
    LAST_RESULTS[0] = res.results
    LAST_RES[0] = res
    out = np.concatenate([res.results[c]["out_slice"] for c in range(NC)],
                         axis=0)
    return out


# revision 6
# speedup vs baseline: 1.1263x; 1.0122x over previous
"""Two-layer GATv2 (heads=1) on 8 Trainium2 NeuronCores — v2.

Per core: nodes dst-sharded (6250/core, 50 blocks of 125). Edges grouped
128/group, batch-major: per batch of 5 blocks, [lo-src groups | hi-src
groups] (lo/hi split because dma_gather indices are int16).

Per group:
  - xl[src] via batched dma_gather (bf16, elem 256B)
  - v = onehot_ea^T @ xr_aug + I @ xlg accumulated in PSUM (bf16 matmuls,
    onehot+ea host-built in fp8, streamed from DRAM)
  - Prelu amortized over 4 (L1) / 8 (L2) groups per PSUM bank
  - logits via DVE STT with accum_out; exp once per batch
  - scatter via one-hot ups matmuls accumulating U|s per block in PSUM
Weights replicated; xl tables AllGathered (bf16) between phases.
"""
import sys

sys.path.insert(0, '/opt/trn_rl_repo')

import numpy as np
import ml_dtypes
from contextlib import ExitStack

import concourse.bass as bass
import concourse.tile as tile
import concourse.bacc as bacc
from concourse import mybir
from concourse.bass_utils import run_bass_kernel_spmd

F32 = mybir.dt.float32
BF16 = mybir.dt.bfloat16
FP8 = mybir.dt.float8e4
I16 = mybir.dt.int16
AF = mybir.ActivationFunctionType
ALU = mybir.AluOpType

NPBF16 = ml_dtypes.bfloat16
NPFP8 = ml_dtypes.float8_e4m3

NEG_SLOPE = 0.2

# Problem geometry (nn_Affinity_GAT_75557064671579)
N = 50000
E = 800000
F_IN = 128
H = 128
OUT = 64
NC = 8
NLOC = N // NC          # 6250
BLK = 125
NB = NLOC // BLK        # 50 blocks
BPB = 5                 # blocks per batch
NBAT = NB // BPB        # 10 batches
HALF = 32768            # int16 index split
PAD_DST = 999.0

LAST_EXEC_NS = [None]
LAST_RESULTS = [None]
LAST_RES = [None]


def prep_inputs(x, edge_index, edge_attr):
    """Host-side graph prep. Returns per-core input dicts + layout meta."""
    src = edge_index[0].astype(np.int64)
    dst = edge_index[1].astype(np.int64)
    ea = edge_attr[:, 0].astype(np.float32)

    # add_self_loops (fill_value='mean' over incoming edges)
    cnt = np.bincount(dst, minlength=N).astype(np.float32)
    asum = np.bincount(dst, weights=ea, minlength=N).astype(np.float32)
    loop_attr = asum / np.maximum(cnt, 1.0)
    src_all = np.concatenate([src, np.arange(N, dtype=np.int64)])
    dst_all = np.concatenate([dst, np.arange(N, dtype=np.int64)])
    ea_all = np.concatenate([ea, loop_attr])

    core_of = dst_all // NLOC
    per_core = []
    nlo = np.zeros((NC, NB), np.int64)
    nhi = np.zeros((NC, NB), np.int64)
    for c in range(NC):
        m = core_of == c
        s_c = src_all[m]
        d_c = dst_all[m] - c * NLOC
        a_c = ea_all[m]
        blk = d_c // BLK
        hi = (s_c >= HALF).astype(np.int64)
        key = blk * 2 + hi
        order = np.lexsort((s_c, key))
        s_c, d_c, a_c, blk, hi, key = (s_c[order], d_c[order], a_c[order],
                                       blk[order], hi[order], key[order])
        # rank within each (blk, hi) bucket
        bc = np.bincount(key, minlength=NB * 2)
        starts = np.concatenate([[0], np.cumsum(bc)])[:-1]
        rank = np.arange(len(s_c)) - starts[key]
        per_core.append((s_c, d_c, a_c, blk, hi, rank))
        nlo[c] = bc[0::2]
        nhi[c] = bc[1::2]

    glo = np.maximum((nlo.max(axis=0) + 127) // 128, 0).astype(np.int64)
    ghi = np.maximum((nhi.max(axis=0) + 127) // 128, 0).astype(np.int64)

    # batch-major group numbering
    glo_bat = np.array([glo[k * BPB:(k + 1) * BPB].sum() for k in range(NBAT)])
    ghi_bat = np.array([ghi[k * BPB:(k + 1) * BPB].sum() for k in range(NBAT)])
    bg = glo_bat + ghi_bat
    bat_goff = np.concatenate([[0], np.cumsum(bg)])
    g_total = int(bat_goff[-1])
    bgmax = int(bg.max())

    # per-block rel group bases within batch
    lo_base = np.zeros(NB, np.int64)   # batch-relative
    hi_base = np.zeros(NB, np.int64)
    for k in range(NBAT):
        off = 0
        for b in range(k * BPB, (k + 1) * BPB):
            lo_base[b] = off
            off += glo[b]
        for b in range(k * BPB, (k + 1) * BPB):
            hi_base[b] = off
            off += ghi[b]

    # idx tensor column offsets (per batch: lo cols then hi cols; 16 idx/col)
    idx_off_lo = np.zeros(NBAT, np.int64)
    idx_off_hi = np.zeros(NBAT, np.int64)
    off = 0
    for k in range(NBAT):
        idx_off_lo[k] = off
        off += glo_bat[k] * 8
        idx_off_hi[k] = off
        off += ghi_bat[k] * 8
    idx_cols = int(off)

    meta = dict(glo=glo, ghi=ghi, glo_bat=glo_bat, ghi_bat=ghi_bat,
                bg=bg, bat_goff=bat_goff, g_total=g_total, bgmax=bgmax,
                lo_base=lo_base, hi_base=hi_base,
                idx_off_lo=idx_off_lo, idx_off_hi=idx_off_hi,
                idx_cols=idx_cols)

    core_inputs = []
    for c in range(NC):
        s_c, d_c, a_c, blk, hi, rank = per_core[c]
        k = blk // BPB
        grel = np.where(hi == 0, lo_base[blk], hi_base[blk]) + rank // 128
        gabs = bat_goff[k] + grel
        p = rank % 128

        onehot = np.zeros((128, g_total * 128), NPFP8)
        col = gabs * 128 + p
        onehot[(d_c - blk * BLK), col] = NPFP8(1.0)
        onehot[127, col] = a_c.astype(NPFP8)

        onehotT = np.zeros((128, g_total * 128), NPFP8)
        onehotT[p, gabs * 128 + (d_c - blk * BLK)] = NPFP8(1.0)

        idx_all = np.zeros((128, idx_cols), np.int16)
        # flat position within the batch's lo (or hi) idx list
        base_in_bat = np.where(hi == 0, lo_base[blk], hi_base[blk] - glo_bat[k])
        j = base_in_bat * 128 + rank
        colpos = np.where(hi == 0, idx_off_lo[k], idx_off_hi[k]) + j // 16
        val = np.where(hi == 0, s_c, s_c - HALF).astype(np.int16)
        idx_all[j % 16, colpos] = val
        # each of the 8 GPSIMD Q7 cores reads its own 16-partition stripe
        idx_all = np.tile(idx_all[0:16], (8, 1))

        xT = np.ascontiguousarray(
            x[c * NLOC:(c + 1) * NLOC].T).astype(NPBF16)
        core_inputs.append(dict(xT_bf=xT, onehot_d=onehot, onehotT_d=onehotT,
                                idx_d=idx_all))
    return core_inputs, meta


def build_program(meta):
    glo, ghi = meta['glo'], meta['ghi']
    glo_bat, ghi_bat = meta['glo_bat'], meta['ghi_bat']
    bg, bat_goff = meta['bg'], meta['bat_goff']
    g_total, bgmax = meta['g_total'], meta['bgmax']
    lo_base, hi_base = meta['lo_base'], meta['hi_base']
    idx_off_lo, idx_off_hi = meta['idx_off_lo'], meta['idx_off_hi']
    idx_cols = meta['idx_cols']

    nc = bacc.Bacc("TRN2", target_bir_lowering=False, debug=False,
                   num_devices=NC, num_swdge_queues=4)

    # --- DRAM I/O ---
    xT_bf = nc.dram_tensor("xT_bf", [F_IN, NLOC], BF16, kind="ExternalInput")
    onehot_d = nc.dram_tensor("onehot_d", [128, g_total * 128], FP8,
                              kind="ExternalInput")
    onehotT_d = nc.dram_tensor("onehotT_d", [128, g_total * 128], FP8,
                               kind="ExternalInput")
    idx_d = nc.dram_tensor("idx_d", [128, idx_cols], I16, kind="ExternalInput")
    Wl1_d = nc.dram_tensor("Wl1b", [F_IN, H], BF16, kind="ExternalInput")
    Wr1_d = nc.dram_tensor("Wr1b", [F_IN, H], BF16, kind="ExternalInput")
    Wl2_d = nc.dram_tensor("Wl2b", [H, OUT], BF16, kind="ExternalInput")
    Wr2_d = nc.dram_tensor("Wr2b", [H, OUT], BF16, kind="ExternalInput")
    We1_d = nc.dram_tensor("We1t", [1, NB * H], BF16, kind="ExternalInput")
    We2_d = nc.dram_tensor("We2t", [1, NB * OUT], BF16, kind="ExternalInput")
    att1_d = nc.dram_tensor("att1b", [128, H], BF16, kind="ExternalInput")
    att2_d = nc.dram_tensor("att2b", [128, OUT], BF16, kind="ExternalInput")
    b1_d = nc.dram_tensor("b1b", [128, H], F32, kind="ExternalInput")
    b2_d = nc.dram_tensor("b2b", [128, OUT], F32, kind="ExternalInput")
    iota_d = nc.dram_tensor("iota_row", [128, 128], BF16, kind="ExternalInput")
    identb_d = nc.dram_tensor("identb", [128, 128], BF16, kind="ExternalInput")
    ident8_d = nc.dram_tensor("ident8", [128, 128], FP8, kind="ExternalInput")
    ones_d = nc.dram_tensor("ones_colb", [128, 1], BF16, kind="ExternalInput")
    out_slice = nc.dram_tensor("out_slice", [NLOC, OUT], F32,
                               kind="ExternalOutput")

    # internal DRAM
    ag1_in = nc.dram_tensor("ag1_in", [NLOC, H], BF16)
    xl1_full = nc.dram_tensor("xl1_full", [N, H], BF16, addr_space="Shared")
    ag2_in = nc.dram_tensor("ag2_in", [NLOC, 128], BF16)
    xl2_full = nc.dram_tensor("xl2_full", [N, 128], BF16, addr_space="Shared")

    with tile.TileContext(nc) as tc:
        with ExitStack() as ctx:
            cpool = ctx.enter_context(tc.tile_pool(name="const", bufs=1))
            mm_pool = ctx.enter_context(tc.tile_pool(name="mmx", bufs=3))
            mm_ps = ctx.enter_context(tc.tile_pool(name="mmps", bufs=2,
                                                   space="PSUM"))
            ps_t = ctx.enter_context(tc.tile_pool(name="pst", bufs=2,
                                                  space="PSUM"))
            ps_v = ctx.enter_context(tc.tile_pool(name="psv", bufs=2,
                                                  space="PSUM"))
            ps_u = ctx.enter_context(tc.tile_pool(name="psu", bufs=2,
                                                  space="PSUM"))
            z_pool = ctx.enter_context(tc.tile_pool(name="zp", bufs=2))
            scr_pool = ctx.enter_context(tc.tile_pool(name="scr", bufs=2))
            sw_pool = ctx.enter_context(tc.tile_pool(name="swp", bufs=3))
            np_pool = ctx.enter_context(tc.tile_pool(name="nodep", bufs=3))

            def cload(name, dram, shape, dt):
                t = cpool.tile(shape, dt, tag=name)
                nc.sync.dma_start(out=t[:], in_=dram[:])
                return t

            Wl1 = cload("Wl1", Wl1_d, [F_IN, H], BF16)
            Wr1 = cload("Wr1", Wr1_d, [F_IN, H], BF16)
            Wl2 = cload("Wl2", Wl2_d, [H, OUT], BF16)
            Wr2 = cload("Wr2", Wr2_d, [H, OUT], BF16)
            att1 = cload("att1", att1_d, [128, H], BF16)
            att2 = cload("att2", att2_d, [128, OUT], BF16)
            b1 = cload("b1", b1_d, [128, H], F32)
            b2 = cload("b2", b2_d, [128, OUT], F32)
            iota_row = cload("iota", iota_d, [128, 128], BF16)
            identb = cload("identb", identb_d, [128, 128], BF16)
            ident8 = cload("ident8", ident8_d, [128, 128], FP8)
            ones_colb = cload("onesc", ones_d, [128, 1], BF16)
            idx_sb = cpool.tile([128, idx_cols], I16, tag="idxsb")
            nc.sync.dma_start(out=idx_sb[:], in_=idx_d[:])

            # persistent xr tables (row 127 = We, rows 125:127 zero)
            xr1_sb = cpool.tile([128, NB * H], BF16, tag="xr1")
            nc.vector.memset(xr1_sb[:], 0.0)
            nc.sync.dma_start(out=xr1_sb[127:128, :], in_=We1_d[:])
            xr2_sb = cpool.tile([128, NB * OUT], BF16, tag="xr2")
            nc.vector.memset(xr2_sb[:], 0.0)
            nc.sync.dma_start(out=xr2_sb[127:128, :], in_=We2_d[:])

            # edge-phase streaming buffers
            xlg_bufs = [cpool.tile([128, bgmax, 128], BF16, tag=f"xlg{i}",
                                   name=f"xlg{i}") for i in range(3)]
            oh_bufs = [cpool.tile([128, bgmax * 128], FP8, tag=f"oh{i}",
                                  name=f"oh{i}") for i in range(2)]
            ohT_bufs = [cpool.tile([128, bgmax * 128], FP8, tag=f"ohT{i}",
                                   name=f"ohT{i}") for i in range(3)]
            logit_bufs = [cpool.tile([128, bgmax], F32, tag=f"lg{i}",
                                     name=f"lg{i}") for i in range(2)]
            w_bufs = [cpool.tile([128, bgmax], F32, tag=f"wb{i}",
                                 name=f"wb{i}") for i in range(2)]
            wbf_bufs = [cpool.tile([128, bgmax], BF16, tag=f"wf{i}",
                                   name=f"wf{i}") for i in range(2)]

            qn = [0]

            # ============ layer 1 node matmul phase ============
            for j in range(NB):
                xT_t = mm_pool.tile([F_IN, BLK], BF16, tag="xTt")
                nc.sync.dma_start(out=xT_t[:],
                                  in_=xT_bf[:, j * BLK:(j + 1) * BLK])
                pa = mm_ps.tile([BLK, H], F32, tag="mm")
                nc.tensor.matmul(out=pa[:], lhsT=xT_t[:], rhs=Wl1[:],
                                 start=True, stop=True)
                sa = mm_pool.tile([BLK, H], BF16, tag="sa")
                nc.scalar.activation(sa[:], pa[:], AF.Copy)
                nc.sync.dma_start(out=ag1_in[j * BLK:(j + 1) * BLK, :],
                                  in_=sa[:])
                pb = mm_ps.tile([BLK, H], F32, tag="mm")
                nc.tensor.matmul(out=pb[:], lhsT=xT_t[:], rhs=Wr1[:],
                                 start=True, stop=True)
                nc.vector.tensor_copy(out=xr1_sb[0:BLK, j * H:(j + 1) * H],
                                      in_=pb[:])

            tc.strict_bb_all_engine_barrier()
            nc.gpsimd.collective_compute(
                "AllGather", ALU.bypass,
                replica_groups=[list(range(NC))],
                ins=[ag1_in[:]], outs=[xl1_full[:]])
            tc.strict_bb_all_engine_barrier()

            # ============ edge phase ============
            def edge_layer(F, table_d, xr_sb, att_t, b_t, sink, layer2):
                PR = 512 // F

                def emit_dmas(k):
                    g0 = int(bat_goff[k])
                    BG = int(bg[k])
                    glo_k = int(glo_bat[k])
                    ghi_k = int(ghi_bat[k])
                    oh = oh_bufs[k % 2]
                    nc.sync.dma_start(
                        out=oh[:, 0:BG * 128],
                        in_=onehot_d[:, g0 * 128:(g0 + BG) * 128])
                    ohT = ohT_bufs[k % 3]
                    nc.scalar.dma_start(
                        out=ohT[:, 0:BG * 128],
                        in_=onehotT_d[:, g0 * 128:(g0 + BG) * 128])
                    xg = xlg_bufs[k % 3]
                    # each gather split in two halves on separate SWDGE
                    # queues: Q7 emission parallelizes across queues
                    q = 0
                    for base, n_g, coff, tab in (
                            (0, glo_k, int(idx_off_lo[k]), table_d[:]),
                            (glo_k, ghi_k, int(idx_off_hi[k]),
                             table_d[HALF:N, :])):
                        if n_g == 0:
                            continue
                        h = (n_g + 1) // 2
                        for a, z in ((0, h), (h, n_g)):
                            if z <= a:
                                continue
                            nidx = (z - a) * 128
                            nc.gpsimd.dma_gather(
                                xg[:, base + a:base + z, :], tab,
                                idx_sb[:, coff + a * 8:coff + z * 8],
                                nidx, nidx, 128,
                                queue_num=q % 4, single_packet=False)
                            q += 1

                def blk_of_group(k, grel):
                    """Map batch-relative group index -> block id."""
                    glo_k = int(glo_bat[k])
                    if grel < glo_k:
                        off = 0
                        for b in range(k * BPB, (k + 1) * BPB):
                            if grel < off + glo[b]:
                                return b
                            off += int(glo[b])
                    else:
                        off = glo_k
                        for b in range(k * BPB, (k + 1) * BPB):
                            if grel < off + ghi[b]:
                                return b
                            off += int(ghi[b])
                    raise AssertionError("bad group")

                def phase1_tiles(k):
                    """Generator: emits one PR-tile of phase1 per next()."""
                    BG = int(bg[k])
                    xg = xlg_bufs[k % 3]
                    oh = oh_bufs[k % 2]
                    lgb = logit_bufs[k % 2]
                    ntile = (BG + PR - 1) // PR
                    for t in range(ntile):
                        gA = t * PR
                        gZ = min(gA + PR, BG)
                        used = gZ - gA
                        # ONE accumulation group per PSUM bank-tile:
                        # start=True resets has_written BANK-WIDE, so only
                        # the first matmul starts; fresh regions write
                        # (hw=0), repeat writes accumulate (hw=1).
                        vt = ps_v.tile([128, 512], F32, tag="vt")
                        for i in range(used):
                            g = gA + i
                            b = blk_of_group(k, g)
                            nc.tensor.matmul(
                                out=vt[:, i * F:(i + 1) * F],
                                lhsT=oh[:, g * 128:(g + 1) * 128],
                                rhs=xr_sb[:, b * F:(b + 1) * F],
                                start=(i == 0), stop=False,
                                skip_group_check=True)
                            nc.tensor.matmul(
                                out=vt[:, i * F:(i + 1) * F],
                                lhsT=ident8[:],
                                rhs=xg[:, g, 0:F],
                                start=False, stop=(i == used - 1),
                                skip_group_check=True)
                        zt = z_pool.tile([128, 512], BF16, tag="zt")
                        nc.scalar.activation(zt[:, 0:used * F],
                                             vt[:, 0:used * F],
                                             AF.Prelu, alpha=NEG_SLOPE)
                        for i in range(used):
                            g = gA + i
                            scr = scr_pool.tile([128, F], BF16, tag="sc")
                            nc.vector.scalar_tensor_tensor(
                                out=scr[:], in0=zt[:, i * F:(i + 1) * F],
                                scalar=1.0, in1=att_t[:, 0:F],
                                op0=ALU.mult, op1=ALU.mult,
                                accum_out=lgb[:, g:g + 1])
                        yield

                def emit_exp(k):
                    BG = int(bg[k])
                    nc.scalar.activation(w_bufs[k % 2][:, 0:BG],
                                         logit_bufs[k % 2][:, 0:BG], AF.Exp)
                    nc.scalar.activation(wbf_bufs[k % 2][:, 0:BG],
                                         w_bufs[k % 2][:, 0:BG], AF.Copy)

                def phase2_block(k, b):
                    """ups accumulation + node phase + sink for block b."""
                    xg = xlg_bufs[k % 3]
                    ohT = ohT_bufs[k % 3]
                    wb = w_bufs[k % 2]
                    wf = wbf_bufs[k % 2]
                    glo_k = int(glo_bat[k])
                    groups = (list(range(int(lo_base[b]),
                                         int(lo_base[b] + glo[b]))) +
                              list(range(int(hi_base[b]),
                                         int(hi_base[b] + ghi[b]))))
                    # ONE accumulation group per bank: start only on the
                    # very first matmul; s column writes fresh (hw=0) then
                    # accumulates. L2 rhs 0:F+1 includes the table's ones
                    # col; L1 adds a separate N=1 s-matmul vs ones.
                    ups = ps_u.tile([128, F + 1], F32, tag="ups")
                    ng = len(groups)
                    for gi, g in enumerate(groups):
                        xlgw = sw_pool.tile([128, F], BF16, tag="xw")
                        nc.scalar.activation(xlgw[:], xg[:, g, 0:F], AF.Copy,
                                             scale=wb[:, g:g + 1])
                        nc.tensor.matmul(out=ups[:, 0:F],
                                         lhsT=ohT[:, g * 128:(g + 1) * 128],
                                         rhs=xlgw[:],
                                         start=(gi == 0), stop=False,
                                         skip_group_check=True)
                        nc.tensor.matmul(out=ups[:, F:F + 1],
                                         lhsT=ohT[:, g * 128:(g + 1) * 128],
                                         rhs=wf[:, g:g + 1],
                                         start=False, stop=(gi == ng - 1),
                                         skip_group_check=True)
                    # node phase: h = elu(U/s + b) = max(y, exp(min(y,0))-1)
                    rs = np_pool.tile([BLK, 1], F32, tag="rs")
                    nc.vector.reciprocal(rs[:], ups[0:BLK, F:F + 1])
                    y = np_pool.tile([BLK, F], BF16 if not layer2 else F32,
                                     tag="y")
                    nc.vector.scalar_tensor_tensor(
                        out=y[:], in0=ups[0:BLK, 0:F], scalar=rs[:],
                        in1=b_t[0:BLK, 0:F], op0=ALU.mult, op1=ALU.add)
                    mn = np_pool.tile([BLK, F], BF16 if not layer2 else F32,
                                      tag="mn")
                    nc.scalar.activation(mn[:], y[:], AF.Relu, scale=-1.0)
                    ex = np_pool.tile([BLK, F], BF16 if not layer2 else F32,
                                      tag="ex")
                    nc.scalar.activation(ex[:], mn[:], AF.Exp, scale=-1.0)
                    h2 = np_pool.tile([BLK, F], BF16 if not layer2 else F32,
                                      tag="h2")
                    nc.vector.scalar_tensor_tensor(
                        out=h2[:], in0=ex[:], scalar=-1.0, in1=y[:],
                        op0=ALU.add, op1=ALU.max)
                    sink(b, h2)

                # --- software pipeline over batches ---
                emit_dmas(0)
                prev_blocks = []
                for k in range(NBAT):
                    if k + 1 < NBAT:
                        emit_dmas(k + 1)
                    pending = list(prev_blocks)
                    tiles = phase1_tiles(k)
                    nt = (int(bg[k]) + PR - 1) // PR
                    ti = 0
                    for tile_i in tiles:
                        ti += 1
                        # interleave one phase2 block roughly every nt/BPB
                        if pending and ti % max(1, nt // BPB) == 0:
                            kb, bb = pending.pop(0)
                            phase2_block(kb, bb)
                    emit_exp(k)
                    for kb, bb in pending:
                        phase2_block(kb, bb)
                    prev_blocks = [(k, b)
                                   for b in range(k * BPB, (k + 1) * BPB)]
                for kb, bb in prev_blocks:
                    phase2_block(kb, bb)

            # ---- layer 1 ----
            def sink1(b, h2):
                hp = ps_t.tile([H, BLK], BF16, tag="tr")
                nc.tensor.matmul(out=hp[:], lhsT=h2[:, :],
                                 rhs=identb[0:BLK, 0:BLK], is_transpose=True)
                hs = np_pool.tile([H, BLK], BF16, tag="hs")
                nc.scalar.activation(hs[:], hp[:], AF.Copy)
                pa = mm_ps.tile([BLK, OUT], F32, tag="mm")
                nc.tensor.matmul(out=pa[:], lhsT=hs[:], rhs=Wl2[:],
                                 start=True, stop=True)
                sa = mm_pool.tile([BLK, OUT + 1], BF16, tag="sa2")
                nc.vector.memset(sa[:, OUT:OUT + 1], 1.0)
                nc.scalar.activation(sa[:, 0:OUT], pa[:], AF.Copy)
                nc.scalar.dma_start(
                    out=ag2_in[b * BLK:(b + 1) * BLK, 0:OUT + 1], in_=sa[:])
                pb = mm_ps.tile([BLK, OUT], F32, tag="mm")
                nc.tensor.matmul(out=pb[:], lhsT=hs[:], rhs=Wr2[:],
                                 start=True, stop=True)
                nc.vector.tensor_copy(
                    out=xr2_sb[0:BLK, b * OUT:(b + 1) * OUT], in_=pb[:])

            edge_layer(H, xl1_full, xr1_sb, att1, b1, sink1, layer2=False)

            tc.strict_bb_all_engine_barrier()
            nc.gpsimd.collective_compute(
                "AllGather", ALU.bypass,
                replica_groups=[list(range(NC))],
                ins=[ag2_in[:]], outs=[xl2_full[:]])
            tc.strict_bb_all_engine_barrier()

            # ---- layer 2 ----
            def sink2(b, h2):
                nc.scalar.dma_start(out=out_slice[b * BLK:(b + 1) * BLK, :],
                                    in_=h2[:])

            edge_layer(OUT, xl2_full, xr2_sb, att2, b2, sink2, layer2=True)

    nc.compile()
    return nc


_CACHE = {}


def kernel(x, edge_index, edge_attr, Wl1, Wr1, We1, att1, b1,
           Wl2, Wr2, We2, att2, b2, _trace=False):
    x = np.asarray(x, np.float32)
    edge_index = np.asarray(edge_index)
    edge_attr = np.asarray(edge_attr, np.float32)

    core_inputs, meta = prep_inputs(x, edge_index, edge_attr)

    key = (meta['g_total'], tuple(meta['glo'].tolist()),
           tuple(meta['ghi'].tolist()))
    if key not in _CACHE:
        _CACHE[key] = build_program(meta)
    nc = _CACHE[key]

    consts = dict(
        Wl1b=np.asarray(Wl1, np.float32).astype(NPBF16),
        Wr1b=np.asarray(Wr1, np.float32).astype(NPBF16),
        Wl2b=np.asarray(Wl2, np.float32).astype(NPBF16),
        Wr2b=np.asarray(Wr2, np.float32).astype(NPBF16),
        We1t=np.tile(np.asarray(We1, np.float32).reshape(1, H),
                     (1, NB)).astype(NPBF16),
        We2t=np.tile(np.asarray(We2, np.float32).reshape(1, OUT),
                     (1, NB)).astype(NPBF16),
        att1b=np.tile(np.asarray(att1, np.float32)[None, :],
                      (128, 1)).astype(NPBF16),
        att2b=np.tile(np.asarray(att2, np.float32)[None, :],
                      (128, 1)).astype(NPBF16),
        b1b=np.tile(np.asarray(b1, np.float32)[None, :], (128, 1)),
        b2b=np.tile(np.asarray(b2, np.float32)[None, :], (128, 1)),
        iota_row=np.tile(np.arange(128, dtype=np.float32)[None, :],
                         (128, 1)).astype(NPBF16),
        identb=np.eye(128, dtype=np.float32).astype(NPBF16),
        ident8=np.eye(128, dtype=np.float32).astype(NPFP8),
        ones_colb=np.ones((128, 1), NPBF16),
    )
    in_maps = []
    for c in range(NC):
        m = dict(consts)
        m.update(core_inputs[c])
        in_maps.append(m)

    res = run_bass_kernel_spmd(nc, in_maps, list(range(NC)), trace=_trace)
    LAST_EXEC_NS[0] = res.exec_time_ns
    LAST_RESULTS[0] = res.results
    LAST_RES[0] = res
    out = np.concatenate([res.results[c]["out_slice"] for c in range(NC)],
                         axis=0)
    return out
